# revision 11
# baseline (speedup 1.0000x reference)
"""Trainium2 Bass kernel for ClusterMemoryAMP cross-entropy loss.

Computes: loss = 0.5*(ce(hard_logits) + ce(mean_logits)) where
logits = normalize(inputs) @ features.T / 0.05, split in halves of 50000.

Sharding: feature bank [100000, 256] row-sharded across 8 cores
(12500 rows each; cores 0-3 own the "mean" half, 4-7 the "hard" half).
Each core computes its logits slab, fused exp+row-sum (distributed
log-softmax partials), and its locally-owned target logits via
indirect-DMA gather. Host combines the tiny per-core partials.
"""

import math

import numpy as np
import orjson

import concourse.bass as bass
import concourse.mybir as mybir
import concourse.tile as tile
from concourse.bass_utils import run_bass_kernel_spmd
from concourse.masks import make_identity

# Problem constants (hardcoded per harness contract)
B = 1024  # batch
D = 256  # feature dim
NC = 50000  # clusters per half
M = 8  # cores
ROWS = NC * 2 // M  # 12500 feature rows per core
NPAD = 300  # zero-padded columns per core slab
NCOLS = ROWS + NPAD  # 12800 = 25 * 512
TEMP = 0.05

P = 128
JT = B // P  # 8 batch chunks
KS = D // P  # 2 contraction chunks
MMN = 512  # matmul moving free dim
GW = 2048  # psum group width (4 banks)
GROUPS = [(c0, min(GW, NCOLS - c0)) for c0 in range(0, NCOLS, GW)]
NGRP = len(GROUPS)  # 7: six 2048-wide + one 512-wide

F32 = mybir.dt.float32
F32R = mybir.dt.float32r
I32 = mybir.dt.int32

_NC_CACHE = None


def _split_multiwait_json(raw: bytes) -> bytes:
    """The walrus build in this container only supports one sync-wait per
    instruction; Tile emits multi-wait instructions (e.g. the tail drain).
    Hoist all-but-the-last wait onto single-wait NoOps on the same engine."""
    m = orjson.loads(raw)
    k = 0
    for f in m["functions"]:
        for bb in f["blocks"]:
            out = []
            for ins in bb["instructions"]:
                si = ins.get("sync_info")
                waits = (si or {}).get("on_wait") or []
                if len(waits) > 1:
                    for w in waits[:-1]:
                        k += 1
                        out.append(
                            {
                                "engine": ins["engine"],
                                "ins": [],
                                "name": f"{ins['name']}-sw{k}",
                                "opcode": "NoOp",
                                "outs": [],
                                "sync_info": {"on_wait": [w], "on_update": []},
                            }
                        )
                    si["on_wait"] = [waits[-1]]
                out.append(ins)
            bb["instructions"] = out
    return orjson.dumps(m)


def _install_json_fix(nc):
    orig = nc.to_json_bytes
    nc.to_json_bytes = lambda: _split_multiwait_json(orig())
    return nc


def _build_nc():
    nc = bass.Bass()

    x_d = nc.dram_tensor("x", [B, D], F32, kind="ExternalInput")
    wt_d = nc.dram_tensor("wt", [D, NCOLS], F32R, kind="ExternalInput")
    wg_d = nc.dram_tensor("wg", [ROWS, D], F32, kind="ExternalInput")
    tidx_d = nc.dram_tensor("tidx", [P, JT], I32, kind="ExternalInput")
    tmask_d = nc.dram_tensor("tmask", [P, JT], F32, kind="ExternalInput")
    osum_d = nc.dram_tensor("osum", [P, JT], F32, kind="ExternalOutput")
    otgt_d = nc.dram_tensor("otgt", [P, JT], F32, kind="ExternalOutput")

    with tile.TileContext(nc) as tc:
        with (
            tc.tile_pool(name="const", bufs=1) as const,
            tc.tile_pool(name="scratch", bufs=2) as scratch,
            tc.tile_pool(name="wpool", bufs=3) as wpool,
            tc.tile_pool(name="epool", bufs=2) as epool,
            tc.tile_pool(name="psum", bufs=2, space="PSUM") as psum,
        ):
            # ---- Phase 0: load x, normalize rows, fold in 1/TEMP ----
            xs = const.tile([P, JT, D], F32, tag="xs")  # x rows, b = j*128+p
            nc.sync.dma_start(xs[:], x_d.rearrange("(j p) d -> p j d", p=P))

            norms = const.tile([P, JT], F32, tag="norms")
            for j in range(JT):
                sq = scratch.tile([P, D], F32, tag="sq")
                nc.vector.tensor_tensor(
                    sq[:], xs[:, j], xs[:, j], mybir.AluOpType.mult
                )
                nc.vector.reduce_sum(
                    norms[:, j : j + 1], sq[:], axis=mybir.AxisListType.X
                )
            # scale_b = exp(-0.5*ln(|x|^2) + ln(1/TEMP)) = 1/(TEMP*|x|)
            lnn = const.tile([P, JT], F32, tag="lnn")
            nc.scalar.activation(lnn[:], norms[:], mybir.ActivationFunctionType.Ln)
            bias_t = const.tile([P, 1], F32, tag="bias")
            nc.vector.memset(bias_t[:], math.log(1.0 / TEMP))
            scal = const.tile([P, JT], F32, tag="scal")
            nc.scalar.activation(
                scal[:],
                lnn[:],
                mybir.ActivationFunctionType.Exp,
                bias=bias_t[:],
                scale=-0.5,
            )
            for j in range(JT):
                nc.vector.tensor_scalar_mul(xs[:, j], xs[:, j], scal[:, j : j + 1])

            # ---- Phase 0b: transpose x_scaled -> xT [128, KS, B] ----
            ident = const.tile([P, P], F32, tag="ident")
            make_identity(nc, ident[:])
            xT = const.tile([P, KS, B], F32R, tag="xT")
            for j in range(JT):
                for s in range(KS):
                    pt = psum.tile([P, GW], F32, tag="pg")
                    nc.tensor.transpose(
                        pt[:, :P], xs[:, j, s * P : (s + 1) * P], ident[:]
                    )
                    nc.vector.tensor_copy(xT[:, s, j * P : (j + 1) * P], pt[:, :P])

            # ---- Phase 0c: gather target rows, compute target logits ----
            tidx = const.tile([P, JT], I32, tag="tidx")
            nc.sync.dma_start(tidx[:], tidx_d[:])
            tmask = const.tile([P, JT], F32, tag="tmask")
            nc.sync.dma_start(tmask[:], tmask_d[:])
            tl = const.tile([P, JT], F32, tag="tl")
            for j in range(JT):
                g = scratch.tile([P, D], F32, tag="g")
                nc.gpsimd.indirect_dma_start(
                    out=g[:],
                    out_offset=None,
                    in_=wg_d[:, :],
                    in_offset=bass.IndirectOffsetOnAxis(
                        ap=tidx[:, j : j + 1], axis=0
                    ),
                )
                prod = scratch.tile([P, D], F32, tag="prod")
                nc.vector.tensor_tensor(
                    prod[:], g[:], xs[:, j], mybir.AluOpType.mult
                )
                nc.vector.reduce_sum(
                    tl[:, j : j + 1], prod[:], axis=mybir.AxisListType.X
                )
            nc.vector.tensor_tensor(tl[:], tl[:], tmask[:], mybir.AluOpType.mult)
            nc.sync.dma_start(otgt_d[:], tl[:])

            # ---- Main loop: logits matmul + fused exp/row-sum ----
            wt_r = wt_d.rearrange("(s p) c -> p s c", p=P)
            sums_g = const.tile([P, JT, NGRP], F32, tag="sums_g")
            for gi, (c0, w) in enumerate(GROUPS):
                wtile = wpool.tile([P, KS, GW], F32R, tag="wt")
                nc.sync.dma_start(wtile[:, :, :w], wt_r[:, :, c0 : c0 + w])
                for j in range(JT):
                    pg = psum.tile([P, GW], F32, tag="pg")
                    for t in range(w // MMN):
                        for s in range(KS):
                            nc.tensor.matmul(
                                pg[:, t * MMN : (t + 1) * MMN],
                                lhsT=xT[:, s, j * P : (j + 1) * P],
                                rhs=wtile[:, s, t * MMN : (t + 1) * MMN],
                                start=(s == 0),
                                stop=(s == KS - 1),
                            )
                    ex = epool.tile([P, GW], F32, tag="ex")
                    nc.scalar.activation(
                        ex[:, :w],
                        pg[:, :w],
                        mybir.ActivationFunctionType.Exp,
                        accum_out=sums_g[:, j, gi : gi + 1],
                    )

            sums = const.tile([P, JT], F32, tag="sums")
            nc.vector.reduce_sum(sums[:], sums_g[:], axis=mybir.AxisListType.X)
            nc.sync.dma_start(osum_d[:], sums[:])

    return _install_json_fix(nc)


def _get_nc():
    global _NC_CACHE
    if _NC_CACHE is None:
        _NC_CACHE = _build_nc()
    return _NC_CACHE


def _prep_in_maps(inputs, targets, features):
    x = np.ascontiguousarray(np.asarray(inputs, dtype=np.float32))
    t = np.asarray(targets).astype(np.int64)
    feats = np.asarray(features, dtype=np.float32)

    in_maps = []
    for c in range(M):
        half = c // (M // 2)  # 0 = mean half, 1 = hard half
        ci = c % (M // 2)
        r0 = half * NC + ci * ROWS
        slab = feats[r0 : r0 + ROWS]  # [12500, 256]
        wt = np.zeros((D, NCOLS), dtype=np.float32)
        wt[:, :ROWS] = slab.T
        local = t - ci * ROWS  # target row within this core's slab (per half)
        owned = (local >= 0) & (local < ROWS)
        tidx = np.where(owned, local, 0).astype(np.int32)
        tmask = owned.astype(np.float32)
        # b = j*128 + p -> sbuf [p, j]
        tidx2 = np.ascontiguousarray(tidx.reshape(JT, P).T)
        tmask2 = np.ascontiguousarray(tmask.reshape(JT, P).T)
        in_maps.append(
            {
                "x": x,
                "wt": wt,
                "wg": np.ascontiguousarray(slab),
                "tidx": tidx2,
                "tmask": tmask2,
            }
        )
    return in_maps


def _combine(results):
    """results: list of 8 dicts with osum/otgt [128, 8] -> scalar loss."""

    def flat(a):  # [p, j] -> [b] with b = j*128+p
        return np.asarray(a).T.reshape(-1)

    ces = []
    for half in range(2):
        cores = range(half * (M // 2), (half + 1) * (M // 2))
        s = np.zeros(B, dtype=np.float64)
        tlog = np.zeros(B, dtype=np.float64)
        for c in cores:
            s += flat(results[c]["osum"]).astype(np.float64) - NPAD
            tlog += flat(results[c]["otgt"]).astype(np.float64)
        ces.append(np.mean(np.log(s) - tlog))
    # halves: 0 = mean, 1 = hard; loss = 0.5*(ce(hard)+ce(mean))
    return np.float32(0.5 * (ces[0] + ces[1]))


LAST_RESULT = None  # BassKernelResults of the most recent run (for profiling)


def kernel(inputs, targets, features):
    global LAST_RESULT
    nc = _get_nc()
    in_maps = _prep_in_maps(inputs, targets, features)
    res = run_bass_kernel_spmd(nc, in_maps, core_ids=list(range(M)))
    LAST_RESULT = res
    return _combine(res.results)


# revision 12
# speedup vs baseline: 556.1619x; 556.1619x over previous
"""Trainium2 Bass kernel for ClusterMemoryAMP cross-entropy loss.

Computes: loss = 0.5*(ce(hard_logits) + ce(mean_logits)) where
logits = normalize(inputs) @ features.T / 0.05, split in halves of 50000.

Sharding: feature bank [100000, 256] row-sharded across 8 cores
(12500 rows each; cores 0-3 own the "mean" half, 4-7 the "hard" half).
Each core computes its logits slab, fused exp+row-sum (distributed
log-softmax partials), and its locally-owned target logits via
indirect-DMA gather. Host combines the tiny per-core partials.
"""

import math

import numpy as np
import orjson

import concourse.bass as bass
import concourse.mybir as mybir
import concourse.tile as tile
from concourse.bass_utils import run_bass_kernel_spmd
from concourse.masks import make_identity

# Problem constants (hardcoded per harness contract)
B = 1024  # batch
D = 256  # feature dim
NC = 50000  # clusters per half
M = 8  # cores
ROWS = NC * 2 // M  # 12500 feature rows per core
NPAD = 300  # zero-padded columns per core slab
NCOLS = ROWS + NPAD  # 12800 = 25 * 512
TEMP = 0.05

P = 128
JT = B // P  # 8 batch chunks
KS = D // P  # 2 contraction chunks
MMN = 512  # matmul moving free dim
GW = 2048  # psum group width (4 banks)
GROUPS = [(c0, min(GW, NCOLS - c0)) for c0 in range(0, NCOLS, GW)]
NGRP = len(GROUPS)  # 7: six 2048-wide + one 512-wide

F32 = mybir.dt.float32
F32R = mybir.dt.float32r
I32 = mybir.dt.int32

_NC_CACHE = None


def _split_multiwait_json(raw: bytes) -> bytes:
    """The walrus build in this container only supports one sync-wait per
    instruction; Tile emits multi-wait instructions (e.g. the tail drain).
    Hoist all-but-the-last wait onto single-wait NoOps on the same engine."""
    m = orjson.loads(raw)
    k = 0
    for f in m["functions"]:
        for bb in f["blocks"]:
            out = []
            for ins in bb["instructions"]:
                si = ins.get("sync_info")
                waits = (si or {}).get("on_wait") or []
                if len(waits) > 1:
                    for w in waits[:-1]:
                        k += 1
                        out.append(
                            {
                                "engine": ins["engine"],
                                "ins": [],
                                "name": f"{ins['name']}-sw{k}",
                                "opcode": "NoOp",
                                "outs": [],
                                "sync_info": {"on_wait": [w], "on_update": []},
                            }
                        )
                    si["on_wait"] = [waits[-1]]
                out.append(ins)
            bb["instructions"] = out
    return orjson.dumps(m)


def _install_json_fix(nc):
    orig = nc.to_json_bytes
    nc.to_json_bytes = lambda: _split_multiwait_json(orig())
    return nc


def _build_nc(repeat: int = 1):
    nc = bass.Bass()

    x_d = nc.dram_tensor("x", [B, D], F32, kind="ExternalInput")
    wt_d = nc.dram_tensor("wt", [D, NCOLS], F32R, kind="ExternalInput")
    wg_d = nc.dram_tensor("wg", [ROWS, D], F32, kind="ExternalInput")
    tidx_d = nc.dram_tensor("tidx", [P, JT], I32, kind="ExternalInput")
    tmask_d = nc.dram_tensor("tmask", [P, JT], F32, kind="ExternalInput")
    osum_d = nc.dram_tensor("osum", [P, JT], F32, kind="ExternalOutput")
    otgt_d = nc.dram_tensor("otgt", [P, JT], F32, kind="ExternalOutput")

    with tile.TileContext(nc) as tc:
        with (
            tc.tile_pool(name="const", bufs=1) as const,
            tc.tile_pool(name="scratch", bufs=2) as scratch,
            tc.tile_pool(name="wpool", bufs=3) as wpool,
            tc.tile_pool(name="epool", bufs=2) as epool,
            tc.tile_pool(name="psum", bufs=2, space="PSUM") as psum,
        ):
          for _rep in range(repeat):
            # ---- Phase 0: load x, normalize rows, fold in 1/TEMP ----
            xs = const.tile([P, JT, D], F32, tag="xs")  # x rows, b = j*128+p
            nc.sync.dma_start(xs[:], x_d.rearrange("(j p) d -> p j d", p=P))

            norms = const.tile([P, JT], F32, tag="norms")
            for j in range(JT):
                sq = scratch.tile([P, D], F32, tag="sq")
                nc.vector.tensor_tensor(
                    sq[:], xs[:, j], xs[:, j], mybir.AluOpType.mult
                )
                nc.vector.reduce_sum(
                    norms[:, j : j + 1], sq[:], axis=mybir.AxisListType.X
                )
            # scale_b = exp(-0.5*ln(|x|^2) + ln(1/TEMP)) = 1/(TEMP*|x|)
            lnn = const.tile([P, JT], F32, tag="lnn")
            nc.scalar.activation(lnn[:], norms[:], mybir.ActivationFunctionType.Ln)
            bias_t = const.tile([P, 1], F32, tag="bias")
            nc.vector.memset(bias_t[:], math.log(1.0 / TEMP))
            scal = const.tile([P, JT], F32, tag="scal")
            nc.scalar.activation(
                scal[:],
                lnn[:],
                mybir.ActivationFunctionType.Exp,
                bias=bias_t[:],
                scale=-0.5,
            )
            for j in range(JT):
                nc.vector.tensor_scalar_mul(xs[:, j], xs[:, j], scal[:, j : j + 1])

            # ---- Phase 0b: transpose x_scaled -> xT [128, KS, B] ----
            ident = const.tile([P, P], F32, tag="ident")
            make_identity(nc, ident[:])
            xT = const.tile([P, KS, B], F32R, tag="xT")
            for j in range(JT):
                for s in range(KS):
                    pt = psum.tile([P, GW], F32, tag="pg")
                    nc.tensor.transpose(
                        pt[:, :P], xs[:, j, s * P : (s + 1) * P], ident[:]
                    )
                    nc.vector.tensor_copy(xT[:, s, j * P : (j + 1) * P], pt[:, :P])

            # ---- Phase 0c: gather target rows, compute target logits ----
            tidx = const.tile([P, JT], I32, tag="tidx")
            nc.sync.dma_start(tidx[:], tidx_d[:])
            tmask = const.tile([P, JT], F32, tag="tmask")
            nc.sync.dma_start(tmask[:], tmask_d[:])
            tl = const.tile([P, JT], F32, tag="tl")
            for j in range(JT):
                g = scratch.tile([P, D], F32, tag="g")
                nc.gpsimd.indirect_dma_start(
                    out=g[:],
                    out_offset=None,
                    in_=wg_d[:, :],
                    in_offset=bass.IndirectOffsetOnAxis(
                        ap=tidx[:, j : j + 1], axis=0
                    ),
                )
                prod = scratch.tile([P, D], F32, tag="prod")
                nc.vector.tensor_tensor(
                    prod[:], g[:], xs[:, j], mybir.AluOpType.mult
                )
                nc.vector.reduce_sum(
                    tl[:, j : j + 1], prod[:], axis=mybir.AxisListType.X
                )
            nc.vector.tensor_tensor(tl[:], tl[:], tmask[:], mybir.AluOpType.mult)
            nc.sync.dma_start(otgt_d[:], tl[:])

            # ---- Main loop: logits matmul + fused exp/row-sum ----
            wt_r = wt_d.rearrange("(s p) c -> p s c", p=P)
            sums_g = const.tile([P, JT, NGRP], F32, tag="sums_g")
            for gi, (c0, w) in enumerate(GROUPS):
                wtile = wpool.tile([P, KS, GW], F32R, tag="wt")
                nc.sync.dma_start(wtile[:, :, :w], wt_r[:, :, c0 : c0 + w])
                for j in range(JT):
                    pg = psum.tile([P, GW], F32, tag="pg")
                    for t in range(w // MMN):
                        for s in range(KS):
                            nc.tensor.matmul(
                                pg[:, t * MMN : (t + 1) * MMN],
                                lhsT=xT[:, s, j * P : (j + 1) * P],
                                rhs=wtile[:, s, t * MMN : (t + 1) * MMN],
                                start=(s == 0),
                                stop=(s == KS - 1),
                            )
                    ex = epool.tile([P, GW], F32, tag="ex")
                    nc.scalar.activation(
                        ex[:, :w],
                        pg[:, :w],
                        mybir.ActivationFunctionType.Exp,
                        accum_out=sums_g[:, j, gi : gi + 1],
                    )

            sums = const.tile([P, JT], F32, tag="sums")
            nc.vector.reduce_sum(sums[:], sums_g[:], axis=mybir.AxisListType.X)
            nc.sync.dma_start(osum_d[:], sums[:])

    return _install_json_fix(nc)


def _get_nc():
    global _NC_CACHE
    if _NC_CACHE is None:
        _NC_CACHE = _build_nc()
    return _NC_CACHE


def _prep_in_maps(inputs, targets, features):
    x = np.ascontiguousarray(np.asarray(inputs, dtype=np.float32))
    t = np.asarray(targets).astype(np.int64)
    feats = np.asarray(features, dtype=np.float32)

    in_maps = []
    for c in range(M):
        half = c // (M // 2)  # 0 = mean half, 1 = hard half
        ci = c % (M // 2)
        r0 = half * NC + ci * ROWS
        slab = feats[r0 : r0 + ROWS]  # [12500, 256]
        wt = np.zeros((D, NCOLS), dtype=np.float32)
        wt[:, :ROWS] = slab.T
        local = t - ci * ROWS  # target row within this core's slab (per half)
        owned = (local >= 0) & (local < ROWS)
        tidx = np.where(owned, local, 0).astype(np.int32)
        tmask = owned.astype(np.float32)
        # b = j*128 + p -> sbuf [p, j]
        tidx2 = np.ascontiguousarray(tidx.reshape(JT, P).T)
        tmask2 = np.ascontiguousarray(tmask.reshape(JT, P).T)
        in_maps.append(
            {
                "x": x,
                "wt": wt,
                "wg": np.ascontiguousarray(slab),
                "tidx": tidx2,
                "tmask": tmask2,
            }
        )
    return in_maps


def _combine(results):
    """results: list of 8 dicts with osum/otgt [128, 8] -> scalar loss."""

    def flat(a):  # [p, j] -> [b] with b = j*128+p
        return np.asarray(a).T.reshape(-1)

    ces = []
    for half in range(2):
        cores = range(half * (M // 2), (half + 1) * (M // 2))
        s = np.zeros(B, dtype=np.float64)
        tlog = np.zeros(B, dtype=np.float64)
        for c in cores:
            s += flat(results[c]["osum"]).astype(np.float64) - NPAD
            tlog += flat(results[c]["otgt"]).astype(np.float64)
        ces.append(np.mean(np.log(s) - tlog))
    # halves: 0 = mean, 1 = hard; loss = 0.5*(ce(hard)+ce(mean))
    return np.float32(0.5 * (ces[0] + ces[1]))


LAST_RESULT = None  # BassKernelResults of the most recent run (for profiling)


def kernel(inputs, targets, features):
    global LAST_RESULT
    nc = _get_nc()
    in_maps = _prep_in_maps(inputs, targets, features)
    res = run_bass_kernel_spmd(nc, in_maps, core_ids=list(range(M)))
    LAST_RESULT = res
    return _combine(res.results)
